# revision 1
# baseline (speedup 1.0000x reference)
"""Trainium2 Bass kernel for nn_CommunicationLayer (gnn_message_passing).

Computes, for A=3 agents over batch B with feature dim D=128:
    total       = sum_a x_a                      # [1, B, D]
    mean_others = (total - x_i) / (A-1)          # [A, B, D]
    out_i       = x_i + mean_others_i @ W + b    # [A, B, D]

Rewritten as   out_i = x_i + sum_{j != i} x_j @ (W/(A-1)) + b
so the whole computation is 3 accumulating matmuls per batch tile plus one
residual add; no total/mean tensors are ever materialized.

Distribution: data-parallel over the batch axis across 8 NeuronCores
(no cross-device communication), weights replicated.

Per-core dataflow (batch-major chunks of 2048 rows, 3 MiB loads with 8 KiB
contiguous runs per partition):
  DMA in (SP/HWDGE)
    -> PE transpose (f32r, 1.5 cyc/row) -> ACT copy PSUM->SBUF
    -> 3x f32r matmul per 128-row group, rhs = [W'|W'] (N=256 -> 1 cyc/row)
       accumulating into strided PSUM column blocks (per-element
       has_written handles the overlapping block pattern)
    -> DVE tensor_add (residual x_i from the exact fp32 view + PSUM
       evacuation, fused in one op)
    -> per-half-chunk DMA out on the otherwise-idle GPSIMD sequencer, so
       the SP load stream never blocks behind store data dependencies.
"""

import numpy as np

import concourse.bacc as bacc
import concourse.bass as bass  # noqa: F401
import concourse.mybir as mybir
from concourse.tile import TileContext
from concourse.masks import make_identity
from concourse.bass_utils import run_bass_kernel_spmd

A = 3
B = 524288
D = 128
NCORES = 8
BC = B // NCORES          # 65536 batch rows per core
CHUNK = 2048              # batch rows per chunk
W_PER = CHUNK // 128      # 16 rows per partition per chunk
NCHUNK = BC // CHUNK      # 32
NQUAD = W_PER // 4        # 4 quads of 4 groups per chunk

F32 = mybir.dt.float32
F32R = mybir.dt.float32r


def build_bass():
    # Bacc (not plain Bass): its compile pipeline moves matmul waits onto
    # ldweights and splits >1-wait sync conditions into event semaphores,
    # which the fused 4-byte matmuls need to pass walrus codegen.
    nc = bacc.Bacc(None, target_bir_lowering=False)

    # x is declared float32r so the PE transpose + matmul path runs at the
    # f32r rate (1.5 / 1.0 cycles per row vs 2 / 4 for fp32). The bytes are
    # plain fp32; the DVE residual add reads them through a float32 bitcast,
    # so the residual term stays exact.
    x_ext = nc.declare_dram_parameter("x", [A, BC, D], F32R, isOutput=False)
    m_ext = nc.declare_dram_parameter("m", [D, 2 * D], F32, isOutput=False)
    y_ext = nc.declare_dram_parameter("y", [A, BC, D], F32, isOutput=True)

    with TileContext(nc) as tc:
        with (
            tc.tile_pool(name="const", bufs=1) as cpool,
            tc.tile_pool(name="xin_pool", bufs=5) as in_pool,
            tc.tile_pool(name="xout_pool", bufs=4) as out_pool,
            tc.tile_pool(name="xt_pool", bufs=6) as xt_pool,
            tc.tile_pool(name="tpsum_pool", bufs=4, space="PSUM") as tpsum_pool,
            tc.tile_pool(name="mpsum_pool", bufs=4, space="PSUM") as mpsum_pool,
        ):
            ident_f = cpool.tile([128, 128], F32)
            make_identity(nc, ident_f)
            ident = cpool.tile([128, 128], F32R)
            nc.scalar.copy(out=ident, in_=ident_f)

            mw_f = cpool.tile([D, 2 * D], F32)
            nc.sync.dma_start(out=mw_f, in_=m_ext[:, :])
            # Walrus requires f32r matmul operands to be produced as f32r;
            # the ACT copy performs the rounding cast.
            mw_r = cpool.tile([D, 2 * D], F32R)
            nc.scalar.copy(out=mw_r, in_=mw_f)

            for c in range(NCHUNK):
                b0 = c * CHUNK
                xin = in_pool.tile([128, A * CHUNK], F32R, tag="xin")
                src = x_ext[:, b0:b0 + CHUNK, :].rearrange(
                    "a (p w) d -> p a (w d)", p=128
                )
                nc.sync.dma_start(
                    out=xin.rearrange("p (a f) -> p a f", a=A), in_=src
                )

                # fp32 view of xin for the (exact) DVE residual add
                xin4 = xin.bitcast(F32).rearrange("p (a w d) -> p a w d", a=A, d=D)

                for h in range(2):
                    # Per-half-chunk output tile: its store DMA (issued on
                    # the otherwise-idle GPSIMD sequencer) waits only on this
                    # half's 8 residual adds, so the SP sequencer's load
                    # stream never blocks behind store data dependencies,
                    # and stores start draining early.
                    xoh = out_pool.tile([128, A * 8 * D], F32, tag="xout")
                    xoh4 = xoh.rearrange("p (a w d) -> p a w d", a=A, d=D)
                    for q in range(2 * h, 2 * h + 2):
                        # Transpose 4 groups x 3 agents into feature-major.
                        xts = []
                        for j in range(A):
                            tp = tpsum_pool.tile([128, 512], F32R, tag="tp")
                            for g4 in range(4):
                                g = q * 4 + g4
                                nc.tensor.transpose(
                                    tp[:, g4 * 128:(g4 + 1) * 128],
                                    xin[:, j * CHUNK + g * 128:
                                        j * CHUNK + (g + 1) * 128],
                                    ident,
                                )
                            xt = xt_pool.tile([128, 512], F32R, tag="xt")
                            nc.scalar.copy(out=xt, in_=tp)
                            xts.append(xt)

                        for g4 in range(4):
                            g = q * 4 + g4
                            ps = mpsum_pool.tile([128, A * D], F32, tag="ps")
                            ps_r = ps.rearrange("p (i d) -> p i d", d=D)
                            # agent j contributes x_j @ W' to blocks i != j
                            mm_outs = [
                                ps_r[:, 1:3, :],    # j=0 -> blocks 1,2
                                ps_r[:, 0::2, :],   # j=1 -> blocks 0,2
                                ps_r[:, 0:2, :],    # j=2 -> blocks 0,1
                            ]
                            for j in range(A):
                                nc.tensor.matmul(
                                    mm_outs[j],
                                    lhsT=xts[j][:, g4 * 128:(g4 + 1) * 128],
                                    rhs=mw_r,
                                    start=(j == 0),
                                    stop=(j == A - 1),
                                    skip_group_check=True,
                                )
                            # Fused residual add + PSUM->SBUF evacuation.
                            nc.vector.tensor_add(
                                out=xoh4[:, :, g - 8 * h, :],
                                in0=ps_r,
                                in1=xin4[:, :, g, :],
                            )

                    dst = y_ext[:, b0:b0 + CHUNK, :].rearrange(
                        "a (p w) d -> p a w d", p=128
                    )[:, :, 8 * h:8 * h + 8, :]
                    nc.gpsimd.dma_start(out=dst, in_=xoh4)

    # Bacc defers register allocation to its compile() pass (run by
    # finalize); the PJRT exec path serializes nc as-is, so finalize here.
    nc.finalize()
    return nc


def run(inputs, trace=False):
    """Build, compile, and run on 8 cores. Returns (full_output, results_obj)."""
    agent_states = np.asarray(inputs["agent_states"], dtype=np.float32)
    W = np.asarray(inputs["W"], dtype=np.float32)
    b = np.asarray(inputs["b"], dtype=np.float32)

    wp = (W * (1.0 / (A - 1))).astype(np.float32)
    m_host = np.ascontiguousarray(np.concatenate([wp, wp], axis=1))

    nc = build_bass()

    in_maps = []
    for i in range(NCORES):
        shard = np.ascontiguousarray(agent_states[:, i * BC:(i + 1) * BC, :])
        in_maps.append({"x": shard, "m": m_host})

    res = run_bass_kernel_spmd(nc, in_maps, list(range(NCORES)), trace=trace)

    out = np.concatenate([r["y"] for r in res.results], axis=1)
    if np.any(b):
        out = out + b.reshape(1, 1, D)
    return out, res


def kernel(**inputs):
    out, _ = run(inputs, trace=False)
    return out



# revision 5
# speedup vs baseline: 2.1291x; 2.1291x over previous
"""Trainium2 Bass kernel for nn_CommunicationLayer (gnn_message_passing).

Computes, for A=3 agents over batch B with feature dim D=128:
    total       = sum_a x_a                      # [1, B, D]
    mean_others = (total - x_i) / (A-1)          # [A, B, D]
    out_i       = x_i + mean_others_i @ W + b    # [A, B, D]

Rewritten with W' = W/(A-1), S = sum_j x_j:
    out_i = x_i @ (I - W') + S @ W'
so PSUM accumulates the COMPLETE output (residual folded into the I-W'
matmul) and a single cast-copy evacuates it.

The 2e-2 rel-err gate leaves ~50x headroom over bf16 rounding (~4e-3),
so all HBM traffic is bf16 — half the bytes of the f32 baseline, which
was already DMA-bound at ~98% duty.

Layout: the host pre-transposes each core's shard to feature-major
x^T [A, D, BC] bf16. On device the batch axis is the free/moving dim:
  - no PE transposes at all (the f32 baseline spent 1/3 of PE on them)
  - both matmul stationaries are the tiny 128x128 weights
  - DMA descriptors are CC*2 = 16 KiB contiguous runs both directions
    (vs 8 KiB loads / 4 KiB stores before), cutting per-descriptor
    overhead on the 16 DMA engines.

Distribution: data-parallel over the batch axis across 8 NeuronCores,
weights replicated, no cross-device communication.

Per-core dataflow (chunks of CC=8192 batch columns):
  SP/HWDGE load x^T chunk [128, 3*CC] bf16
    -> per 512-col block: DVE computes S = x0+x1+x2 (bf16)
    -> PE: psum_i = (I-W')^T-matmul(x_i) + W'^T-matmul(S), f32 psum,
       one 2 KiB bank per agent, 512 moving cols per instruction
    -> evacuate psum -> bf16 out tile (agents 0,1 on ACT, agent 2 on DVE)
    -> Pool/SWDGE store y^T chunk.
Host casts/transposes back to [A, B, D] f32.
"""

import numpy as np
import ml_dtypes

import concourse.bacc as bacc
import concourse.bass as bass  # noqa: F401
import concourse.mybir as mybir
from concourse.tile import TileContext
from concourse.bass_utils import run_bass_kernel_spmd

A = 3
B = 524288
D = 128
NCORES = 8
BC = B // NCORES          # 65536 batch columns per core
CC = 8192                 # batch columns per chunk
NCHUNK = BC // CC         # 8
NBLK = CC // 512          # 16 moving blocks per chunk

F32 = mybir.dt.float32
BF16 = mybir.dt.bfloat16
NPBF16 = ml_dtypes.bfloat16


def build_bass():
    nc = bacc.Bacc(None, target_bir_lowering=False)

    # x/y are feature-major per agent: [A, D, BC]
    x_ext = nc.declare_dram_parameter("x", [A, D, BC], BF16, isOutput=False)
    m_ext = nc.declare_dram_parameter("m", [D, 2 * D], BF16, isOutput=False)
    y_ext = nc.declare_dram_parameter("y", [A, D, BC], BF16, isOutput=True)

    with TileContext(nc) as tc:
        with (
            tc.tile_pool(name="const", bufs=1) as cpool,
            tc.tile_pool(name="xin_pool", bufs=2) as in_pool,
            tc.tile_pool(name="xout_pool", bufs=2) as out_pool,
            tc.tile_pool(name="s_pool", bufs=4) as s_pool,
            tc.tile_pool(name="ps_pool", bufs=8, space="PSUM") as ps_pool,
        ):
            # m[:, 0:128] = I - W', m[:, 128:256] = W'   (lhsT layout:
            # [feat_in partitions, feat_out free], so numpy [fi, fo] as-is)
            mw = cpool.tile([D, 2 * D], BF16)
            nc.sync.dma_start(out=mw, in_=m_ext[:, :])
            m_iw = mw[:, 0:D]
            m_w = mw[:, D:2 * D]

            for c in range(NCHUNK):
                c0 = c * CC
                xin = in_pool.tile([128, A * CC], BF16, tag="xin")
                src = x_ext[:, :, c0:c0 + CC].rearrange("a d c -> d a c")
                nc.sync.dma_start(
                    out=xin.rearrange("p (a c) -> p a c", a=A), in_=src
                )

                xout = out_pool.tile([128, A * CC], BF16, tag="xout")

                for blk in range(NBLK):
                    o = blk * 512
                    xb = [xin[:, i * CC + o:i * CC + o + 512] for i in range(A)]

                    sb = s_pool.tile([128, 512], BF16, tag="s")
                    nc.vector.tensor_add(out=sb, in0=xb[0], in1=xb[1])
                    nc.vector.tensor_add(out=sb, in0=sb, in1=xb[2])

                    # psum_i accumulates the full out_i^T block; issue the
                    # three I-W' matmuls back-to-back, then the three W'
                    # ones, so the stationary only swaps twice per block.
                    ps = [ps_pool.tile([128, 512], F32, tag="ps", name=f"ps{i}")
                          for i in range(A)]
                    for i in range(A):
                        nc.tensor.matmul(ps[i], lhsT=m_iw, rhs=xb[i],
                                         start=True, stop=False)
                    for i in range(A):
                        nc.tensor.matmul(ps[i], lhsT=m_w, rhs=sb,
                                         start=False, stop=True)

                    # Evacuate psum -> bf16 out tile, split across ACT/DVE.
                    for i in range(A):
                        dst = xout[:, i * CC + o:i * CC + o + 512]
                        if i < 2:
                            nc.scalar.copy(out=dst, in_=ps[i])
                        else:
                            nc.vector.tensor_copy(out=dst, in_=ps[i])

                dst = y_ext[:, :, c0:c0 + CC].rearrange("a d c -> d a c")
                nc.gpsimd.dma_start(
                    out=dst, in_=xout.rearrange("p (a c) -> p a c", a=A)
                )

    nc.finalize()
    return nc


def run(inputs, trace=False):
    """Build, compile, and run on 8 cores. Returns (full_output, results_obj)."""
    agent_states = np.asarray(inputs["agent_states"], dtype=np.float32)
    W = np.asarray(inputs["W"], dtype=np.float32)
    b = np.asarray(inputs["b"], dtype=np.float32)

    wp = W * (1.0 / (A - 1))
    m_host = np.concatenate([np.eye(D, dtype=np.float32) - wp, wp],
                            axis=1).astype(NPBF16)

    nc = build_bass()

    # bf16 cast once (contiguous, fast), then per-core feature-major
    # transpose via the uint16 view (generic-dtype strided copy is slower).
    xb16 = agent_states.astype(NPBF16).view(np.uint16)
    in_maps = []
    for i in range(NCORES):
        shard = np.ascontiguousarray(
            xb16[:, i * BC:(i + 1) * BC, :].transpose(0, 2, 1)
        ).view(NPBF16)
        in_maps.append({"x": shard, "m": m_host})

    res = run_bass_kernel_spmd(nc, in_maps, list(range(NCORES)), trace=trace)

    out = np.empty((A, B, D), dtype=np.float32)
    for i in range(NCORES):
        yt = np.asarray(res.results[i]["y"]).astype(np.float32)  # [A, D, BC]
        out[:, i * BC:(i + 1) * BC, :] = yt.transpose(0, 2, 1)
    if np.any(b):
        out += b.reshape(1, 1, D)
    return out, res


def kernel(**inputs):
    out, _ = run(inputs, trace=False)
    return out


# revision 9
# speedup vs baseline: 2.3203x; 1.0898x over previous
"""Trainium2 Bass kernel for nn_CommunicationLayer (gnn_message_passing).

Computes, for A=3 agents over batch B with feature dim D=128:
    total       = sum_a x_a                      # [1, B, D]
    mean_others = (total - x_i) / (A-1)          # [A, B, D]
    out_i       = x_i + mean_others_i @ W + b    # [A, B, D]

Rewritten with W' = W/(A-1), S = sum_j x_j:
    out_i = x_i @ (I - W') + S @ W'
so PSUM accumulates the COMPLETE output (residual folded into the I-W'
matmul) and a single cast-copy evacuates it.

The 2e-2 rel-err gate leaves ~50x headroom over bf16 rounding (~4e-3),
so all HBM traffic is bf16 — half the bytes of the f32 baseline, which
was already DMA-bound at ~98% duty.

Layout: the host pre-transposes each core's shard to feature-major
x^T [A, D, BC] bf16. On device the batch axis is the free/moving dim:
  - no PE transposes at all (the f32 baseline spent 1/3 of PE on them)
  - both matmul stationaries are the tiny 128x128 weights
  - DMA descriptors are CC*2 = 16 KiB contiguous runs both directions
    (vs 8 KiB loads / 4 KiB stores before), cutting per-descriptor
    overhead on the 16 DMA engines.

Distribution: data-parallel over the batch axis across 8 NeuronCores,
weights replicated, no cross-device communication.

Per-core dataflow (chunks of CC=8192 batch columns):
  SP/HWDGE load x^T chunk [128, 3*CC] bf16
    -> per 512-col block: DVE computes S = x0+x1+x2 (bf16)
    -> PE: psum_i = (I-W')^T-matmul(x_i) + W'^T-matmul(S), f32 psum,
       one 2 KiB bank per agent, 512 moving cols per instruction
    -> evacuate psum -> bf16 out tile (agents 0,1 on ACT, agent 2 on DVE)
    -> Pool/SWDGE store y^T chunk.
Host casts/transposes back to [A, B, D] f32.
"""

import numpy as np
import ml_dtypes

import concourse.bacc as bacc
import concourse.bass as bass  # noqa: F401
import concourse.mybir as mybir
from concourse.tile import TileContext
from concourse.bass_utils import run_bass_kernel_spmd

A = 3
B = 524288
D = 128
NCORES = 8
BC = B // NCORES          # 65536 batch columns per core
CC = 16384                # batch columns per chunk (32 KiB DMA runs)
NCHUNK = BC // CC         # 4
NBLK = CC // 512          # 32 moving blocks per chunk

F32 = mybir.dt.float32
BF16 = mybir.dt.bfloat16
NPBF16 = ml_dtypes.bfloat16


def build_bass():
    nc = bacc.Bacc(None, target_bir_lowering=False)

    # x/y are feature-major per agent: [A, D, BC]
    x_ext = nc.declare_dram_parameter("x", [A, D, BC], BF16, isOutput=False)
    m_ext = nc.declare_dram_parameter("m", [D, 2 * D], BF16, isOutput=False)
    y_ext = nc.declare_dram_parameter("y", [A, D, BC], BF16, isOutput=True)

    with TileContext(nc) as tc:
        with (
            tc.tile_pool(name="const", bufs=1) as cpool,
            tc.tile_pool(name="xin_pool", bufs=2) as in_pool,
            tc.tile_pool(name="s_pool", bufs=4) as s_pool,
            tc.tile_pool(name="ps_pool", bufs=8, space="PSUM") as ps_pool,
        ):
            # m[:, 0:128] = I - W', m[:, 128:256] = W'   (lhsT layout:
            # [feat_in partitions, feat_out free], so numpy [fi, fo] as-is)
            mw = cpool.tile([D, 2 * D], BF16)
            nc.sync.dma_start(out=mw, in_=m_ext[:, :])
            m_iw = mw[:, 0:D]
            m_w = mw[:, D:2 * D]

            for c in range(NCHUNK):
                c0 = c * CC
                xin = in_pool.tile([128, A * CC], BF16, tag="xin")
                src = x_ext[:, :, c0:c0 + CC].rearrange("a d c -> d a c")
                nc.sync.dma_start(
                    out=xin.rearrange("p (a c) -> p a c", a=A), in_=src
                )

                for blk in range(NBLK):
                    o = blk * 512
                    xb = [xin[:, i * CC + o:i * CC + o + 512] for i in range(A)]

                    sb = s_pool.tile([128, 512], BF16, tag="s")
                    nc.vector.tensor_add(out=sb, in0=xb[0], in1=xb[1])
                    nc.vector.tensor_add(out=sb, in0=sb, in1=xb[2])

                    # psum_i accumulates the full out_i^T block; issue the
                    # three I-W' matmuls back-to-back, then the three W'
                    # ones, so the stationary only swaps twice per block.
                    ps = [ps_pool.tile([128, 512], F32, tag="ps", name=f"ps{i}")
                          for i in range(A)]
                    for i in range(A):
                        nc.tensor.matmul(ps[i], lhsT=m_iw, rhs=xb[i],
                                         start=True, stop=False)
                    for i in range(A):
                        nc.tensor.matmul(ps[i], lhsT=m_w, rhs=sb,
                                         start=False, stop=True)

                    # Evacuate psum -> bf16 IN PLACE over the consumed x
                    # block (all readers of the region are done), split
                    # across ACT/DVE. Saves an xout pool, which is what
                    # lets CC reach 16384 (32 KiB DMA runs) within SBUF.
                    for i in range(A):
                        dst = xin[:, i * CC + o:i * CC + o + 512]
                        if i < 2:
                            nc.scalar.copy(out=dst, in_=ps[i])
                        else:
                            nc.vector.tensor_copy(out=dst, in_=ps[i])

                dst = y_ext[:, :, c0:c0 + CC].rearrange("a d c -> d a c")
                nc.gpsimd.dma_start(
                    out=dst, in_=xin.rearrange("p (a c) -> p a c", a=A)
                )

    nc.finalize()
    return nc


def run(inputs, trace=False):
    """Build, compile, and run on 8 cores. Returns (full_output, results_obj)."""
    agent_states = np.asarray(inputs["agent_states"], dtype=np.float32)
    W = np.asarray(inputs["W"], dtype=np.float32)
    b = np.asarray(inputs["b"], dtype=np.float32)

    wp = W * (1.0 / (A - 1))
    m_host = np.concatenate([np.eye(D, dtype=np.float32) - wp, wp],
                            axis=1).astype(NPBF16)

    nc = build_bass()

    # bf16 cast once (contiguous, fast), then per-core feature-major
    # transpose via the uint16 view (generic-dtype strided copy is slower).
    xb16 = agent_states.astype(NPBF16).view(np.uint16)
    in_maps = []
    for i in range(NCORES):
        shard = np.ascontiguousarray(
            xb16[:, i * BC:(i + 1) * BC, :].transpose(0, 2, 1)
        ).view(NPBF16)
        in_maps.append({"x": shard, "m": m_host})

    res = run_bass_kernel_spmd(nc, in_maps, list(range(NCORES)), trace=trace)

    out = np.empty((A, B, D), dtype=np.float32)
    for i in range(NCORES):
        yt = np.asarray(res.results[i]["y"]).astype(np.float32)  # [A, D, BC]
        out[:, i * BC:(i + 1) * BC, :] = yt.transpose(0, 2, 1)
    if np.any(b):
        out += b.reshape(1, 1, D)
    return out, res


def kernel(**inputs):
    out, _ = run(inputs, trace=False)
    return out
